# revision 15
# baseline (speedup 1.0000x reference)
"""Trainium2 Bass kernel for linear (taylor/sparse) attention — v2.

Reference computation (per batch b, with xf = x.reshape(b, C, N)):
    Q = Wq@xf + bq            [Cqk, N]
    K = Wk@xf + bk            [Cqk, N]
    V = Wv@xf + bv            [C, N]
    Qh = Q / ||Q||_2 (per position), Kh = K / ||K||_2
    tailor[n]   = 1 / (N + Qh[:,n] . (sum_n Kh + eps))
    out[:, n]   = gamma * tailor[n] * (sum_n V + (Kh @ V^T)^T @ Qh[:,n])

Distribution: 8 cores = 4 batches x 2 halves of N.  The key restructuring vs
v1: V is never materialized per-position.  Each core accumulates a raw factor
against x directly (moving operand = x in n-major layout):
    G' = sum_t S_t^T @ [xT_t | 1],  S_t = [rnorm*Kraw | 1 | rnorm]  [128n, 34]
so G' rows are 0:32 = sum rnorm*Kraw (x) x, 32 = sum x (xsum), 33 = sum
rnorm*x; col 256 = the same against ones (s, N_loc, rho).  K's bias is
deferred via algebra: ||K+bk||^2 = ||Kraw||^2 + 2*x.(Wk^T bk) + ||bk||^2
(the cross term is an extra fused-projection column), and the bk term of the
factor is restored post-collective from the rnorm-weighted sums.  The local
G' is pushed through Wv'^T on-core ([34,256] GEMM), AllGathered pairwise
(35 KB), and the global Mx [34, 257] is assembled with a handful of small
vector ops:
    Mx[0:32] = A'@Wv'^T + bk (x) r' + (Ksum+eps) (x) bv'   (matrix')
    Mx[32]   = Wv' xsum + N bv'                            (value_sum)
    Mx[:,256]= [Ksum + eps ; N]                            (denominator col)
Phase 2 is one GEMM per 128-position tile with stationary Q_aug [33, 128]
(rows 0:32 = Q+bq from a layout-A projection done during the collective gap,
row 32 = ||Q+bq||), then out = num * approx_recip(den), written bf16.
gamma is folded into Wv'/bv' on the host; the host transposes/casts back.
"""

import ml_dtypes
import numpy as np
from contextlib import ExitStack

import concourse.bass as bass
import concourse.bacc as bacc
import concourse.tile as tile
from concourse import mybir
from concourse import bass_utils
from concourse.masks import make_identity

F32 = mybir.dt.float32
BF16 = mybir.dt.bfloat16
ALU = mybir.AluOpType
ACTF = mybir.ActivationFunctionType
AXL = mybir.AxisListType

B, C, HH, WW = 4, 256, 128, 128
N = HH * WW            # 16384 positions per batch
NSH = N // 2           # 8192 positions per core
CQK = 32
PW = 66                # fused projection width: [Q | K | qb | kb]
FR = 34                # factor rows: 32 (A) + xsum row + r row
MXW = 257              # mx width: 256 + denominator col
NT512 = NSH // 512     # 16
NT128 = NSH // 128     # 64
GRP = 8
EPS = 1e-6

_CACHE = {}


def _build():
    nc = bacc.Bacc("TRN2", target_bir_lowering=False, debug=False, num_devices=8)

    xs = nc.dram_tensor("xs", [NT512, 128, 2, 512], BF16, kind="ExternalInput").ap()
    xst = nc.dram_tensor("xst", [NT512, 128, 4, 258], BF16, kind="ExternalInput").ap()
    wqk = nc.dram_tensor("wqk", [C, PW], BF16, kind="ExternalInput").ap()
    wvt = nc.dram_tensor("wvt", [C, C], BF16, kind="ExternalInput").ap()
    bqc = nc.dram_tensor("bqc", [CQK, 1], F32, kind="ExternalInput").ap()
    bkx = nc.dram_tensor("bkx", [FR, 1], F32, kind="ExternalInput").ap()
    bvg = nc.dram_tensor("bvg", [C], F32, kind="ExternalInput").ap()
    nrm2 = nc.dram_tensor("nrm2", [2], F32, kind="ExternalInput").ap()
    out = nc.dram_tensor("out", [NSH, C], BF16, kind="ExternalOutput").ap()

    with tile.TileContext(nc) as tc, ExitStack() as ctx:
        _body(ctx, tc, nc, xs, xst, wqk, wvt, bqc, bkx, bvg, nrm2, out)

    nc.compile()
    return nc


def _body(ctx, tc, nc, xs, xst, wqk, wvt, bqc, bkx, bvg, nrm2, out):
    singles = ctx.enter_context(tc.tile_pool(name="singles", bufs=1))
    xpool = ctx.enter_context(tc.tile_pool(name="x", bufs=NT512))
    xstpool = ctx.enter_context(tc.tile_pool(name="xst", bufs=6))
    sqpool = ctx.enter_context(tc.tile_pool(name="sq", bufs=3))
    smalls = ctx.enter_context(tc.tile_pool(name="smalls", bufs=4))
    outpool = ctx.enter_context(tc.tile_pool(name="outp", bufs=3))

    ps_sh = ctx.enter_context(tc.tile_pool(name="ps_sh", bufs=2, space="PSUM"))
    ps_fac = ctx.enter_context(tc.tile_pool(name="ps_fac", bufs=2, space="PSUM"))
    ps_p2 = ctx.enter_context(tc.tile_pool(name="ps_p2", bufs=2, space="PSUM"))
    dram = ctx.enter_context(tc.tile_pool(name="dram", bufs=1, space="DRAM"))

    # ---- one-time setup ----
    wqk_sb = singles.tile([128, 2, PW], BF16)
    nc.sync.dma_start(wqk_sb[:], wqk.rearrange("(cb cp) w -> cp cb w", cb=2))
    wvt_sb = singles.tile([128, 2, C], BF16)
    nc.sync.dma_start(wvt_sb[:], wvt.rearrange("(cb cp) w -> cp cb w", cb=2))
    bq_col = singles.tile([CQK, 1], F32)
    nc.gpsimd.dma_start(bq_col[:], bqc)
    bkx_col = singles.tile([FR, 1], F32)
    nc.gpsimd.dma_start(bkx_col[:], bkx)
    bv_rep = singles.tile([FR, C], F32)
    nc.gpsimd.dma_start(bv_rep[:], bvg.unsqueeze(0).partition_broadcast(FR).squeeze(1))
    nbk_rep = singles.tile([128, 1], F32)
    nc.gpsimd.dma_start(
        nbk_rep[:], nrm2[0:1].unsqueeze(0).partition_broadcast(128).squeeze(1)
    )
    nbq_rep = singles.tile([128, 1], F32)
    nc.gpsimd.dma_start(
        nbq_rep[:], nrm2[1:2].unsqueeze(0).partition_broadcast(128).squeeze(1)
    )
    ident = singles.tile([128, 128], F32)
    make_identity(nc, ident[:])
    identb = singles.tile([128, 128], BF16)
    make_identity(nc, identb[:])

    kvt_all = singles.tile([128, NT128, PW], BF16)
    kh_all = singles.tile([128, NT128, FR], BF16)
    nc.vector.memset(kh_all[:, :, CQK], 1.0)  # ones col -> G' xsum row
    ssqk = singles.tile([128, 2, NT128], F32)
    rnorm_stack = singles.tile([128, NT128], F32)
    ssq_stack = singles.tile([128, NT128], F32)
    qx = singles.tile([CQK + 1, NSH], BF16)

    gps = [None, None]
    gs_tiles = [None, None]
    fps = None
    xt_tiles = [None] * NT512
    xst_tiles = [None] * NT512

    def emit_xform(a):
        # local pre-collective transform: F~ps accumulates G'^T-chunks @ Wv'^T
        nonlocal fps
        gs = singles.tile([FR, C + 1], BF16, tag=f"gs{a}")
        gs_tiles[a] = gs
        nc.scalar.copy(gs[:], gps[a][:])
        tp = ps_sh.tile([128, 2, FR], BF16, tag="shared")
        nc.tensor.transpose(tp[:, 0, :], gs[:, 0:128], identb[0:FR, 0:FR])
        nc.tensor.transpose(tp[:, 1, :], gs[:, 128:256], identb[0:FR, 0:FR])
        gt = singles.tile([128, 2, FR], BF16, tag=f"gt{a}")
        nc.vector.tensor_copy(gt[:], tp[:])
        if a == 0:
            fps = ps_fac.tile([FR, C], F32, tag="facs")
        nc.tensor.matmul(
            fps[:], gt[:, 0, :], wvt_sb[:, 0, :], start=(a == 0), stop=False
        )
        nc.tensor.matmul(
            fps[:], gt[:, 1, :], wvt_sb[:, 1, :], start=False, stop=(a == 1)
        )

    # ---- phase 1 ----
    for j in range(NT512):
        xt = xpool.tile([128, 2, 512], BF16)
        nc.sync.dma_start(xt[:], xs[j])
        xt_tiles[j] = xt
        xst4 = xstpool.tile([128, 4, 258], BF16)
        nc.sync.dma_start(xst4[:], xst[j])
        xst_tiles[j] = xst4

        for u in range(4):
            t = j * 4 + u
            # fused [Q | K | qb | kb] projection, n-major: [128, 66]
            pskqv = ps_sh.tile([128, PW], F32, tag="shared")
            for cb in range(2):
                nc.tensor.matmul(
                    pskqv[:], xt[:, cb, u * 128 : (u + 1) * 128],
                    wqk_sb[:, cb, :],
                    start=(cb == 0), stop=(cb == 1),
                )
            nc.scalar.copy(kvt_all[:, t, :], pskqv[:])
            sq = sqpool.tile([128, 2, CQK], BF16)
            nc.gpsimd.tensor_tensor(
                sq[:],
                kvt_all[:, t, 0 : 2 * CQK].rearrange("p (g c) -> p g c", g=2),
                kvt_all[:, t, 0 : 2 * CQK].rearrange("p (g c) -> p g c", g=2),
                ALU.mult,
            )
            nc.vector.tensor_reduce(ssqk[:, :, t], sq[:], axis=AXL.X, op=ALU.add)

            if t % GRP == GRP - 1:
                g0 = t - (GRP - 1)
                gsl = slice(g0, g0 + GRP)
                sskg = smalls.tile([128, GRP], F32)
                nc.vector.scalar_tensor_tensor(
                    sskg[:], kvt_all[:, gsl, PW - 1], 2.0, ssqk[:, 1, gsl],
                    ALU.mult, ALU.add,
                )
                nrmk = smalls.tile([128, GRP], F32)
                nc.scalar.activation(nrmk[:], sskg[:], ACTF.Sqrt, bias=nbk_rep[:])
                nc.vector.reciprocal_approx_fast(rnorm_stack[:, gsl], nrmk[:])
                nc.vector.scalar_tensor_tensor(
                    ssq_stack[:, gsl], kvt_all[:, gsl, PW - 2], 2.0, ssqk[:, 0, gsl],
                    ALU.mult, ALU.add,
                )
                nc.vector.tensor_tensor(
                    kh_all[:, gsl, 0:CQK],
                    kvt_all[:, gsl, CQK : 2 * CQK],
                    rnorm_stack[:, gsl].unsqueeze(2).broadcast_to([128, GRP, CQK]),
                    ALU.mult,
                )
                nc.vector.tensor_copy(kh_all[:, gsl, CQK + 1], rnorm_stack[:, gsl])
                for tt in range(g0, g0 + GRP):
                    a = tt // (NT128 // 2)
                    st = tt % (NT128 // 2) == 0
                    sp = tt % (NT128 // 2) == (NT128 // 2) - 1
                    if st:
                        gps[a] = ps_fac.tile(
                            [FR, C + 1], F32, tag="facs", name=f"gps{a}"
                        )
                    nc.tensor.matmul(
                        gps[a][:, 0 : C + 1], kh_all[:, tt, :],
                        xst_tiles[tt // 4][:, tt % 4, 0 : C + 1],
                        start=st, stop=sp,
                    )
                if t == NT128 // 2 - 1:
                    emit_xform(0)
                elif t == NT128 - 1:
                    emit_xform(1)

    # ---- factor assembly + AllGather ----
    fsb = singles.tile([FR, C + 1], F32)
    nc.vector.tensor_copy(fsb[:, 0:C], fps[:])
    nc.vector.tensor_tensor(
        fsb[:, C : C + 1], gs_tiles[0][:, C : C + 1], gs_tiles[1][:, C : C + 1],
        ALU.add,
    )
    cc_in = dram.tile([FR, C + 1], F32)
    cc_out = dram.tile([2 * FR, C + 1], F32)
    nc.sync.dma_start(cc_in[:], fsb[:])
    nc.gpsimd.collective_compute(
        "AllGather",
        ALU.bypass,
        replica_groups=[[0, 1], [2, 3], [4, 5], [6, 7]],
        ins=[cc_in.opt()],
        outs=[cc_out.opt()],
    )

    # ---- gap work: layout-A Q projection (qx rows 0:32) + ||Q|| row ----
    for j in range(NT512):
        psq = ps_sh.tile([CQK, 512], F32, tag="shared")
        for cb in range(2):
            nc.tensor.matmul(
                psq[:], wqk_sb[:, cb, 0:CQK], xt_tiles[j][:, cb, :],
                start=(cb == 0), stop=(cb == 1),
            )
        if j % 2 == 0:
            nc.scalar.activation(
                qx[0:CQK, j * 512 : (j + 1) * 512], psq[:],
                ACTF.Identity, bias=bq_col[:], scale=1.0,
            )
        else:
            nc.vector.tensor_scalar_add(
                qx[0:CQK, j * 512 : (j + 1) * 512], psq[:], bq_col[:]
            )
    nq = singles.tile([128, NT128], F32)
    nc.scalar.activation(nq[:], ssq_stack[:], ACTF.Sqrt, bias=nbq_rep[:])
    pst = ps_sh.tile([NT128, 128], F32, tag="shared")
    nc.tensor.transpose(pst[:], nq[:], ident[:])
    trT = singles.tile([NT128, 128], BF16)
    nc.vector.tensor_copy(trT[:], pst[:])
    row_scratch = dram.tile([NT128, 128], BF16)
    nc.sync.dma_start(row_scratch[:], trT[:])
    nc.sync.dma_start(
        qx[CQK : CQK + 1, :],
        row_scratch[:].rearrange("a b -> (a b)").unsqueeze(0),
    )

    # ---- post-collective: global factor -> Mx [34, 257] ----
    fac2 = singles.tile([FR, 2, C + 1], F32)
    nc.sync.dma_start(fac2[:], cc_out[:].rearrange("(r p) f -> p r f", r=2))
    facg = singles.tile([FR, C + 1], F32)
    nc.vector.tensor_tensor(facg[:], fac2[:, 0, :], fac2[:, 1, :], ALU.add)
    # dense f32 matmul burst (~4us) after the collective to re-warm the PE
    # clock (HAM) before the phase-2 matmul stream
    for w in range(10):
        wps = ps_fac.tile([128, C + 1], F32, tag="facs", name=f"wps{w}")
        nc.tensor.matmul(
            wps[:], facg[:, 0:128], facg[:], start=True, stop=True
        )
    rrep = singles.tile([FR, 2, C + 1], F32)
    for h in range(2):
        nc.sync.dma_start(
            rrep[:, h, :],
            cc_out[:][FR - 1 + FR * h, :]
            .unsqueeze(0)
            .partition_broadcast(FR)
            .squeeze(1),
        )
    rsum = singles.tile([FR, C + 1], F32)
    nc.vector.tensor_tensor(rsum[:], rrep[:, 0, :], rrep[:, 1, :], ALU.add)
    tmp = singles.tile([FR, C], F32)
    nc.vector.scalar_tensor_tensor(
        tmp[:], rsum[:, 0:C], bkx_col[:], facg[:, 0:C], ALU.mult, ALU.add
    )
    ksn = singles.tile([FR, 1], F32)
    nc.vector.scalar_tensor_tensor(
        ksn[:], rsum[:, C : C + 1], bkx_col[:], facg[:, C : C + 1], ALU.mult, ALU.add
    )
    ksne = singles.tile([FR, 1], F32)
    nc.vector.tensor_scalar_add(ksne[:], ksn[:], EPS)
    mx = singles.tile([FR, MXW], BF16)
    nc.vector.scalar_tensor_tensor(
        mx[:, 0:C], bv_rep[:], ksne[:], tmp[:], ALU.mult, ALU.add
    )
    nc.vector.tensor_copy(mx[:, C : C + 1], ksne[:])

    # ---- phase 2 ----
    out4 = out.rearrange("(t4 u p) c -> t4 p u c", u=4, p=128)
    for t4 in range(NT128 // 4):
        ot = outpool.tile([128, 4, C], BF16)
        for v in range(2):
            ps2 = ps_p2.tile([128, 2, 512], F32)
            for w in range(2):
                t = t4 * 4 + 2 * v + w
                nc.tensor.matmul(
                    ps2[:, w, 0:MXW], qx[:, t * 128 : (t + 1) * 128],
                    mx[0 : CQK + 1, :],
                    start=True, stop=True,
                )
            rden = smalls.tile([128, 2], F32)
            nc.vector.reciprocal_approx_fast(rden[:], ps2[:, :, C])
            for w in range(2):
                u = 2 * v + w
                if u % 2 == 0:
                    nc.vector.tensor_scalar_mul(
                        ot[:, u, :], ps2[:, w, 0:C], rden[:, w : w + 1]
                    )
                else:
                    nc.scalar.mul(ot[:, u, :], ps2[:, w, 0:C], rden[:, w : w + 1])
        nc.sync.dma_start(out4[t4], ot[:])


def _get_nc():
    if "nc" not in _CACHE:
        _CACHE["nc"] = _build()
    return _CACHE["nc"]


def _prep_in_maps(x, Wq, bq, Wk, bk, Wv, bv, gamma):
    g = float(np.asarray(gamma).reshape(-1)[0])
    wqk = np.concatenate(
        [
            Wq.T.astype(np.float32),
            Wk.T.astype(np.float32),
            (Wq.T @ bq.astype(np.float32)).reshape(C, 1),
            (Wk.T @ bk.astype(np.float32)).reshape(C, 1),
        ],
        axis=1,
    ).astype(ml_dtypes.bfloat16)
    wqk = np.ascontiguousarray(wqk)
    wvt = np.ascontiguousarray((g * Wv).T.astype(np.float32)).astype(ml_dtypes.bfloat16)
    bkxv = np.zeros((FR, 1), np.float32)
    bkxv[0:CQK, 0] = bk.astype(np.float32)
    bqc = np.ascontiguousarray(bq.reshape(CQK, 1), dtype=np.float32)
    bvgv = np.ascontiguousarray(g * bv, dtype=np.float32)
    nrm2 = np.array(
        [float(np.sum(bk.astype(np.float64) ** 2)),
         float(np.sum(bq.astype(np.float64) ** 2))],
        dtype=np.float32,
    )

    xf = np.asarray(x, dtype=np.float32).reshape(B, C, N)
    in_maps = []
    for core in range(8):
        b, h = core // 2, core % 2
        xsl = xf[b, :, h * NSH : (h + 1) * NSH]
        xsh = (
            xsl.astype(ml_dtypes.bfloat16)
            .reshape(2, 128, NT512, 512)
            .transpose(2, 1, 0, 3)
        )
        xsh = np.ascontiguousarray(xsh)
        xt2 = np.zeros((NSH, 258), np.float32)
        xt2[:, 0:C] = xsl.T
        xt2[:, C] = 1.0
        xsht = np.ascontiguousarray(
            xt2.astype(ml_dtypes.bfloat16)
            .reshape(NT512, 4, 128, 258)
            .transpose(0, 2, 1, 3)
        )
        in_maps.append(
            {
                "xs": xsh,
                "xst": xsht,
                "wqk": wqk,
                "wvt": wvt,
                "bqc": bqc,
                "bkx": bkxv,
                "bvg": bvgv,
                "nrm2": nrm2,
            }
        )
    return in_maps


def run(inputs, trace=False):
    nc = _get_nc()
    in_maps = _prep_in_maps(**inputs)
    res = bass_utils.run_bass_kernel_spmd(
        nc, in_maps, core_ids=list(range(8)), trace=trace
    )
    outf = np.empty((B, C, N), np.float32)
    for core in range(8):
        b, h = core // 2, core % 2
        outf[b, :, h * NSH : (h + 1) * NSH] = (
            res.results[core]["out"].astype(np.float32).T
        )
    return outf.reshape(B, C, HH, WW), res


def kernel(**inputs):
    out, _ = run(inputs, trace=False)
    return out
